# revision 23
# baseline (speedup 1.0000x reference)
"""Bass/Trainium2 kernel for nn_KernelEdges (gnn_message_passing).

Computes A = exp((g_i + g_j - 2*Xf@Xf.T)/sigma^2) with zeroed diagonal,
broadcast to all B batch slots, where Xf = X.transpose(1,0,2).reshape(N, B*d).

Sharding: rows of the NxN pairwise matrix are split across 8 NeuronCores
(256 rows each).  Each core receives the full transposed operand
XT = Xf.T [B*d, N] (host-prepared, 4 MB), its own column-slice as the
stationary matmul operand, and writes its [B, N/8, N] output slice.

Per-core device work:
  psum[mt,nb] = sum_q XT_q[:, m_slice].T @ XT_q[:, n_block]     (Gram matrix)
              + (-1/2*ones).T @ g_row[n_block]                  (rank-1: -g_j/2)
  A = exp(-2/sigma^2 * psum + g_i/sigma^2)                      (ACT, bias per row)
  DMA A tile to the 8 batch slots of the output.

The diagonal is zeroed on the host (16K elements) after the gather.
"""

import numpy as np

B, N, D = 8, 2048, 64
NCORES = 8
R = N // NCORES          # 256 rows per core
KD = B * D               # 512 contraction dim
NB = 512                 # n-block (one PSUM bank of fp32)
NNB = N // NB            # 4 n-blocks
NMT = R // 128           # 2 m-tiles per core
NQ = KD // 128           # 4 k-tiles

# matmul operand dtype: "f32r" (full-rate fp32 mode, ~4e-4 rel err) or
# "bf16" (half the input bytes + faster PE, ~2e-3 rel err)
MM_MODE = "bf16"


def _build_program(inv_s2):
    import concourse.bass as bass
    import concourse.tile as tile
    from concourse import bacc, mybir

    f32 = mybir.dt.float32
    mm_dt = mybir.dt.bfloat16 if MM_MODE == "bf16" else mybir.dt.float32r

    nc = bacc.Bacc(
        "TRN2", target_bir_lowering=False, debug=False, num_devices=NCORES
    )

    GK = 2 if MM_MODE == "bf16" else 1  # g carried as hi+lo rows in bf16

    xt_d = nc.dram_tensor("xt", [KD, N], mm_dt, kind="ExternalInput").ap()
    lhst_d = nc.dram_tensor("lhst", [KD, R], mm_dt, kind="ExternalInput").ap()
    grow_d = nc.dram_tensor("grow", [GK, N], mm_dt, kind="ExternalInput").ap()
    bias_d = nc.dram_tensor("bias", [128, NMT], f32, kind="ExternalInput").ap()
    out_d = nc.dram_tensor("out", [B, R, N], f32, kind="ExternalOutput").ap()

    with tile.TileContext(nc) as tc:
        with (
            tc.tile_pool(name="persist", bufs=1) as persist,
            tc.tile_pool(name="apool", bufs=1) as apool,
            tc.tile_pool(name="psum", bufs=1, space="PSUM") as pspool,
        ):
            # ---- loads ----
            # all input DMAs go on the scalar (ACT) HWDGE ring so the sync
            # ring is dedicated to output DMAs.
            # tiny tensors first: the rank-1 g_j matmuls depend only on
            # these, so they start during the xt load and warm the PE
            grow_sb = persist.tile([GK, N], mm_dt, name="grow")
            nc.scalar.dma_start(grow_sb[:], grow_d[:])

            bias_sb = persist.tile([128, NMT], f32, name="bias")
            nc.scalar.dma_start(bias_sb[:], bias_d[:])

            neg_half = persist.tile([GK, 128], mm_dt, name="neg_half")
            # -0.5 bit pattern; direct float memset into f32r fails ISA check
            if MM_MODE == "bf16":
                nc.gpsimd.memset(
                    neg_half[:].bitcast(mybir.dt.uint16), 0xBF00
                )
            else:
                nc.gpsimd.memset(
                    neg_half[:].bitcast(mybir.dt.uint32), 0xBF000000
                )

            lhs_sb = persist.tile([128, NQ * R], mm_dt, name="lhs")
            nc.scalar.dma_start(
                lhs_sb[:].rearrange("p (q m) -> p q m", q=NQ),
                lhst_d.rearrange("(q p) m -> p q m", p=128),
            )

            # xt tiles; the last one split in half so the trailing piece
            # (which gates the final matmul batch) is smaller
            NH = N // 2
            xt_sb = []
            for q in range(NQ):
                t = persist.tile([128, N], mm_dt, name=f"xt{q}")
                if q < NQ - 1:
                    nc.scalar.dma_start(t[:], xt_d[q * 128:(q + 1) * 128, :])
                else:
                    for h in range(2):
                        nc.scalar.dma_start(
                            t[:, h * NH:(h + 1) * NH],
                            xt_d[q * 128:(q + 1) * 128, h * NH:(h + 1) * NH],
                        )
                xt_sb.append(t)

            # ---- compute + store ----
            # all 8 accumulation chains live in the 8 PSUM banks at once;
            # chain order: rank-1 (g_j) first, then k-tiles q0..q3 as each
            # xt_q lands, so the PE overlaps the input DMA
            ps = {}
            for mt in range(NMT):
                for nb in range(NNB):
                    ps[mt, nb] = pspool.tile(
                        [128, NB], f32, name=f"ps{mt}{nb}"
                    )
                    nc.tensor.matmul(
                        ps[mt, nb][:],
                        neg_half[:],
                        grow_sb[:, nb * NB:(nb + 1) * NB],
                        start=True,
                        stop=False,
                    )
            a_sb = {
                mt: apool.tile([128, N], f32, name=f"a{mt}")
                for mt in range(NMT)
            }
            for q in range(NQ):
                for h in range(2):
                    for mt in range(NMT):
                        m0 = q * R + mt * 128
                        for nb in range(2 * h, 2 * h + 2):
                            nc.tensor.matmul(
                                ps[mt, nb][:],
                                lhs_sb[:, m0:m0 + 128],
                                xt_sb[q][:, nb * NB:(nb + 1) * NB],
                                start=False,
                                stop=(q == NQ - 1),
                            )
            # ACTs in mt-major order so mt0's output DMA launches as soon
            # as its four n-blocks are done (Scalar executes in FIFO order)
            for mt in range(NMT):
                for nb in range(NNB):
                    nc.scalar.activation(
                        a_sb[mt][:, nb * NB:(nb + 1) * NB],
                        ps[mt, nb][:],
                        mybir.ActivationFunctionType.Exp,
                        bias=bias_sb[:, mt:mt + 1],
                        scale=-2.0 * inv_s2,
                    )
            # one DMA per m-tile replicates [128, 2048] into all 8 batch
            # slots: 8 KB contiguous runs in DRAM
            for mt in range(NMT):
                src = a_sb[mt][:].rearrange(
                    "p (o n) -> p o n", o=1
                ).broadcast_to([128, B, N])
                dst = out_d[
                    :, mt * 128:(mt + 1) * 128, :
                ].rearrange("b p n -> p b n")
                nc.sync.dma_start(dst, src)

    nc.compile()
    return nc


def _prepare(X, log_sigma):
    """Host prep: returns (inv_s2, in_maps) for run_bass_kernel_spmd."""
    X = np.ascontiguousarray(X, dtype=np.float32)
    assert X.shape == (B, N, D), X.shape

    sigma = float(np.exp(np.float32(log_sigma)))
    inv_s2 = 1.0 / (sigma * sigma)

    # XT[b*D+f, n] = X[b, n, f]
    XT = np.ascontiguousarray(X.transpose(0, 2, 1).reshape(KD, N))
    g = np.einsum("kn,kn->n", XT, XT).astype(np.float32)  # [N]
    if MM_MODE == "bf16":
        import ml_dtypes

        XT = np.ascontiguousarray(XT.astype(ml_dtypes.bfloat16))
        g_hi = g.astype(ml_dtypes.bfloat16)
        g_lo = (g - g_hi.astype(np.float32)).astype(ml_dtypes.bfloat16)
        grow_np = np.stack([g_hi, g_lo])  # [2, N]
    else:
        grow_np = g[None, :]

    in_maps = []
    for c in range(NCORES):
        r0 = c * R
        bias_np = np.empty((128, NMT), dtype=np.float32)
        for mt in range(NMT):
            bias_np[:, mt] = g[r0 + mt * 128: r0 + (mt + 1) * 128] * inv_s2
        in_maps.append({
            "xt": XT,
            "lhst": np.ascontiguousarray(XT[:, r0:r0 + R]),
            "grow": grow_np,
            "bias": bias_np,
        })
    return inv_s2, in_maps


def kernel(X, log_sigma):
    from concourse.bass_utils import run_bass_kernel_spmd

    inv_s2, in_maps = _prepare(X, log_sigma)
    nc = _build_program(inv_s2)
    res = run_bass_kernel_spmd(nc, in_maps, list(range(NCORES)))
    out = np.concatenate([res.results[c]["out"] for c in range(NCORES)], axis=1)
    idx = np.arange(N)
    out[:, idx, idx] = 0.0
    return out


# revision 24
# speedup vs baseline: 1.1040x; 1.1040x over previous
"""Bass/Trainium2 kernel for nn_KernelEdges (gnn_message_passing).

Computes A = exp((g_i + g_j - 2*Xf@Xf.T)/sigma^2) with zeroed diagonal,
broadcast to all B batch slots, where Xf = X.transpose(1,0,2).reshape(N, B*d).

Sharding: rows of the NxN pairwise matrix are split across 8 NeuronCores
(256 rows each).  Each core receives the full transposed operand
XT = Xf.T [B*d, N] (host-prepared, 4 MB), its own column-slice as the
stationary matmul operand, and writes its [B, N/8, N] output slice.

Per-core device work:
  psum[mt,nb] = sum_q XT_q[:, m_slice].T @ XT_q[:, n_block]     (Gram matrix)
              + (-1/2*ones).T @ g_row[n_block]                  (rank-1: -g_j/2)
  A = exp(-2/sigma^2 * psum + g_i/sigma^2)                      (ACT, bias per row)
  DMA A tile to the 8 batch slots of the output.

The diagonal is zeroed on the host (16K elements) after the gather.
"""

import numpy as np

B, N, D = 8, 2048, 64
NCORES = 8
R = N // NCORES          # 256 rows per core
KD = B * D               # 512 contraction dim
NB = 512                 # n-block (one PSUM bank of fp32)
NNB = N // NB            # 4 n-blocks
NMT = R // 128           # 2 m-tiles per core
NQ = KD // 128           # 4 k-tiles

# matmul operand dtype: "f32r" (full-rate fp32 mode, ~4e-4 rel err) or
# "bf16" (half the input bytes + faster PE, ~2e-3 rel err)
MM_MODE = "f32r"


def _build_program(inv_s2):
    import concourse.bass as bass
    import concourse.tile as tile
    from concourse import bacc, mybir

    f32 = mybir.dt.float32
    mm_dt = mybir.dt.bfloat16 if MM_MODE == "bf16" else mybir.dt.float32r

    nc = bacc.Bacc(
        "TRN2", target_bir_lowering=False, debug=False, num_devices=NCORES
    )

    GK = 2 if MM_MODE == "bf16" else 1  # g carried as hi+lo rows in bf16

    xt_d = nc.dram_tensor("xt", [KD, N], mm_dt, kind="ExternalInput").ap()
    lhst_d = nc.dram_tensor("lhst", [KD, R], mm_dt, kind="ExternalInput").ap()
    grow_d = nc.dram_tensor("grow", [GK, N], mm_dt, kind="ExternalInput").ap()
    bias_d = nc.dram_tensor("bias", [128, NMT], f32, kind="ExternalInput").ap()
    out_d = nc.dram_tensor("out", [B, R, N], f32, kind="ExternalOutput").ap()

    with tile.TileContext(nc) as tc:
        with (
            tc.tile_pool(name="persist", bufs=1) as persist,
            tc.tile_pool(name="apool", bufs=1) as apool,
            tc.tile_pool(name="psum", bufs=1, space="PSUM") as pspool,
        ):
            # ---- loads ----
            # all input DMAs go on the scalar (ACT) HWDGE ring so the sync
            # ring is dedicated to output DMAs.
            # tiny tensors first: the rank-1 g_j matmuls depend only on
            # these, so they start during the xt load and warm the PE
            grow_sb = persist.tile([GK, N], mm_dt, name="grow")
            nc.scalar.dma_start(grow_sb[:], grow_d[:])

            bias_sb = persist.tile([128, NMT], f32, name="bias")
            nc.scalar.dma_start(bias_sb[:], bias_d[:])

            neg_half = persist.tile([GK, 128], mm_dt, name="neg_half")
            # -0.5 bit pattern; direct float memset into f32r fails ISA check
            if MM_MODE == "bf16":
                nc.gpsimd.memset(
                    neg_half[:].bitcast(mybir.dt.uint16), 0xBF00
                )
            else:
                nc.gpsimd.memset(
                    neg_half[:].bitcast(mybir.dt.uint32), 0xBF000000
                )

            lhs_sb = persist.tile([128, NQ * R], mm_dt, name="lhs")
            nc.scalar.dma_start(
                lhs_sb[:].rearrange("p (q m) -> p q m", q=NQ),
                lhst_d.rearrange("(q p) m -> p q m", p=128),
            )

            # xt tiles; the last one split in half so the trailing piece
            # (which gates the final matmul batch) is smaller
            NH = N // 2
            xt_sb = []
            for q in range(NQ):
                t = persist.tile([128, N], mm_dt, name=f"xt{q}")
                if q < NQ - 1:
                    nc.scalar.dma_start(t[:], xt_d[q * 128:(q + 1) * 128, :])
                else:
                    for h in range(2):
                        nc.scalar.dma_start(
                            t[:, h * NH:(h + 1) * NH],
                            xt_d[q * 128:(q + 1) * 128, h * NH:(h + 1) * NH],
                        )
                xt_sb.append(t)

            # ---- compute + store ----
            # all 8 accumulation chains live in the 8 PSUM banks at once;
            # chain order: rank-1 (g_j) first, then k-tiles q0..q3 as each
            # xt_q lands, so the PE overlaps the input DMA
            ps = {}
            for mt in range(NMT):
                for nb in range(NNB):
                    ps[mt, nb] = pspool.tile(
                        [128, NB], f32, name=f"ps{mt}{nb}"
                    )
                    nc.tensor.matmul(
                        ps[mt, nb][:],
                        neg_half[:],
                        grow_sb[:, nb * NB:(nb + 1) * NB],
                        start=True,
                        stop=False,
                    )
            a_sb = {
                mt: apool.tile([128, N], f32, name=f"a{mt}")
                for mt in range(NMT)
            }
            for q in range(NQ):
                for h in range(2):
                    for mt in range(NMT):
                        m0 = q * R + mt * 128
                        for nb in range(2 * h, 2 * h + 2):
                            nc.tensor.matmul(
                                ps[mt, nb][:],
                                lhs_sb[:, m0:m0 + 128],
                                xt_sb[q][:, nb * NB:(nb + 1) * NB],
                                start=False,
                                stop=(q == NQ - 1),
                            )
            # ACTs in mt-major order so mt0's output DMA launches as soon
            # as its four n-blocks are done (Scalar executes in FIFO order)
            for mt in range(NMT):
                for nb in range(NNB):
                    nc.scalar.activation(
                        a_sb[mt][:, nb * NB:(nb + 1) * NB],
                        ps[mt, nb][:],
                        mybir.ActivationFunctionType.Exp,
                        bias=bias_sb[:, mt:mt + 1],
                        scale=-2.0 * inv_s2,
                    )
            # one DMA per m-tile replicates [128, 2048] into all 8 batch
            # slots: 8 KB contiguous runs in DRAM
            for mt in range(NMT):
                src = a_sb[mt][:].rearrange(
                    "p (o n) -> p o n", o=1
                ).broadcast_to([128, B, N])
                dst = out_d[
                    :, mt * 128:(mt + 1) * 128, :
                ].rearrange("b p n -> p b n")
                nc.sync.dma_start(dst, src)

    nc.compile()
    return nc


def _prepare(X, log_sigma):
    """Host prep: returns (inv_s2, in_maps) for run_bass_kernel_spmd."""
    X = np.ascontiguousarray(X, dtype=np.float32)
    assert X.shape == (B, N, D), X.shape

    sigma = float(np.exp(np.float32(log_sigma)))
    inv_s2 = 1.0 / (sigma * sigma)

    # XT[b*D+f, n] = X[b, n, f]
    XT = np.ascontiguousarray(X.transpose(0, 2, 1).reshape(KD, N))
    g = np.einsum("kn,kn->n", XT, XT).astype(np.float32)  # [N]
    if MM_MODE == "bf16":
        import ml_dtypes

        XT = np.ascontiguousarray(XT.astype(ml_dtypes.bfloat16))
        g_hi = g.astype(ml_dtypes.bfloat16)
        g_lo = (g - g_hi.astype(np.float32)).astype(ml_dtypes.bfloat16)
        grow_np = np.stack([g_hi, g_lo])  # [2, N]
    else:
        grow_np = g[None, :]

    in_maps = []
    for c in range(NCORES):
        r0 = c * R
        bias_np = np.empty((128, NMT), dtype=np.float32)
        for mt in range(NMT):
            bias_np[:, mt] = g[r0 + mt * 128: r0 + (mt + 1) * 128] * inv_s2
        in_maps.append({
            "xt": XT,
            "lhst": np.ascontiguousarray(XT[:, r0:r0 + R]),
            "grow": grow_np,
            "bias": bias_np,
        })
    return inv_s2, in_maps


def kernel(X, log_sigma):
    from concourse.bass_utils import run_bass_kernel_spmd

    inv_s2, in_maps = _prepare(X, log_sigma)
    nc = _build_program(inv_s2)
    res = run_bass_kernel_spmd(nc, in_maps, list(range(NCORES)))
    out = np.concatenate([res.results[c]["out"] for c in range(NCORES)], axis=1)
    idx = np.arange(N)
    out[:, idx, idx] = 0.0
    return out
